# revision 24
# baseline (speedup 1.0000x reference)
"""Trainium2 Bass kernel for a quantized shared-expert MLP (SwiGLU, int8 dynamic quant).

Computation (per reference):
  x [2,4096,4096] f32 -> flatten [8192, 4096] -> bf16
  per-token int8 dynamic quant of x; int8 gemm vs w_gate/w_up (per-channel int8);
  swiglu with +-10 clip -> bf16; per-token requant; int8 gemm vs w_down; f32 out.

Strategy: data-parallel over the 8192 tokens across 8 NeuronCores (1024
tokens/core), weights replicated, no collectives.  Matmuls run in bf16, which
is exact here: quantized values are integers in [-127,127] (exact in bf16).

v2 layout (vs v1): two-group token pipeline so the PE never idles --
[g0 gate/up][g1 gate/up][g0 down][g1 down] with quant / requant / DMA
transposes of each group hidden under the other group's matmuls.  Gate and up
weight blocks are fused into one [128, 32k, 512] stationary stream (single MM
per k-step, N=512, one PSUM bank), host-pre-laid-out so every weight DMA is
contiguous per partition (line rate).  DMA traffic is split over three rings:
weights on Sync (SP), x loads on GpSimd (SWDGE), transposes + output stores on
Scalar (ACT).  s_wdown is folded in on the host after the kernel.
"""

import numpy as np
import ml_dtypes

H = 4096
I = 2048
P = 128
T = 1024           # tokens per core (8192 / 8)
N_CORES = 8
QMAX = 127.0
LIMIT = 10.0
MAGIC = 12582912.0  # 1.5 * 2**23: fp32 add/sub rounds to nearest-even integer

NT = T // P        # 8 token tiles per core
GRP = 2            # token-tile groups (pipeline stages)
GT = NT // GRP     # tiles per group (4)
KH = H // P        # 32 k-tiles for gate/up
KI = I // P        # 16 k-tiles for down
NB2 = 512          # fused gate||up free-dim block (256 gate + 256 up)
NBI = (2 * I) // NB2   # 8 gate/up blocks
NB3 = 256          # down-proj free-dim block
NBH = H // NB3     # 16 down blocks

_CACHE = {}


def _build():
    import concourse.bass as bass
    import concourse.bacc as bacc
    import concourse.mybir as mybir
    from concourse import tile
    from contextlib import ExitStack

    f32 = mybir.dt.float32
    bf16 = mybir.dt.bfloat16
    X = mybir.AxisListType.X
    MAX = mybir.AluOpType.max
    MIN = mybir.AluOpType.min
    MULT = mybir.AluOpType.mult
    ADD = mybir.AluOpType.add
    SUB = mybir.AluOpType.subtract
    Copy = mybir.ActivationFunctionType.Copy
    Silu = mybir.ActivationFunctionType.Silu

    nc = bacc.Bacc("TRN2", target_bir_lowering=False, debug=False)

    x_d = nc.dram_tensor("x", [T, H], bf16, kind="ExternalInput")
    wgu_d = nc.dram_tensor("wgu", [NBI * P, KH * NB2], bf16, kind="ExternalInput")
    wd_d = nc.dram_tensor("wd", [NBH * P, KI * NB3], bf16, kind="ExternalInput")
    sgu_d = nc.dram_tensor("sgu", [NBI, NB2], f32, kind="ExternalInput")
    out_d = nc.dram_tensor("out", [T, H], f32, kind="ExternalOutput")

    HH = H // 2        # phase-1 half tile width (2048)

    with ExitStack() as ctx:
        tc = ctx.enter_context(tile.TileContext(nc))

        sc_p = ctx.enter_context(tc.tile_pool(name="scales", bufs=1))
        sgu_p = ctx.enter_context(tc.tile_pool(name="sgu", bufs=4))
        ep2_p = ctx.enter_context(tc.tile_pool(name="ep2", bufs=2))
        ps2_p = ctx.enter_context(tc.tile_pool(name="ps2", bufs=4,
                                               space=bass.MemorySpace.PSUM))
        wgu_p = ctx.enter_context(tc.tile_pool(name="wgu", bufs=2))

        # per-token-tile scale columns [P, NT]
        mx = sc_p.tile([P, NT], f32, tag="mx")
        mx2 = sc_p.tile([P, 2 * NT], f32, tag="mx2")
        sx = sc_p.tile([P, NT], f32, tag="sx")     # x quant scale
        ix = sc_p.tile([P, NT], f32, tag="ix")     # 1 / sx
        mxi = sc_p.tile([P, NT], f32, tag="mxi")
        si = sc_p.tile([P, NT], f32, tag="si")     # inter quant scale
        ii = sc_p.tile([P, NT], f32, tag="ii")     # 1 / si
        r0 = sc_p.tile([P, NT], f32, tag="r0")
        r1 = sc_p.tile([P, NT], f32, tag="r1")

        def nr_recip(out_col, in_col, mc):
            # out = 1/in with one Newton step (HW seed alone is not enough
            # for exact round() parity with the reference's division)
            nc.vector.reciprocal(r0[:, mc], in_col)
            nc.vector.tensor_tensor(r1[:, mc], in_col, r0[:, mc], op=MULT)
            nc.vector.tensor_scalar(r1[:, mc], r1[:, mc], -1.0, 2.0, op0=MULT, op1=ADD)
            nc.vector.tensor_tensor(out_col, r0[:, mc], r1[:, mc], op=MULT)

        # Pool stack discipline (LIFO): [ctx..., inter, qT1, qT0, ph1].
        # ph1+qT0 pop right after group-0 gate/up; the late pools
        # (rq, qiT, wd, outp) push into that freed space.
        # inter and qiT tiles are shared between groups via tags: group 1's
        # allocation of the same tag waits for group 0's release, which the
        # pipeline order satisfies naturally.
        inter_ctx = ExitStack()
        inter_pool = inter_ctx.enter_context(tc.tile_pool(name="inter", bufs=1))
        inter = [[None] * GT, [None] * GT]
        qT_ctx = [ExitStack(), ExitStack()]
        # push qT1 first, then qT0 (qT0 pops earlier)
        qT_pool = [None, None]
        qT_pool[1] = qT_ctx[1].enter_context(tc.tile_pool(name="qT1", bufs=1))
        qT_pool[0] = qT_ctx[0].enter_context(tc.tile_pool(name="qT0", bufs=1))
        qT = [[qT_pool[g].tile([P, KH, P], bf16, tag=f"qT{g}_{j}",
                               name=f"qT{g}_{j}") for j in range(GT)]
              for g in range(GRP)]

        ph1_ctx = ExitStack()
        ph1 = ph1_ctx.enter_context(tc.tile_pool(name="ph1", bufs=2))

        def quant_tile(g, j):
            """Load token tile m, dynamic-quant, transpose into qT[g][j]."""
            m = g * GT + j
            mc = slice(m, m + 1)
            xh = [None, None]
            for h in range(2):
                xh[h] = ph1.tile([P, HH], bf16, tag=f"xh{h}", name=f"xh{m}_{h}")
                nc.scalar.dma_start(xh[h][:], x_d[m * P:(m + 1) * P,
                                                  h * HH:(h + 1) * HH])
                nc.vector.tensor_reduce(mx2[:, 2 * m + h:2 * m + h + 1], xh[h][:],
                                        axis=X, op=MAX, apply_absolute_value=True)
            nc.vector.tensor_reduce(mx[:, mc], mx2[:, 2 * m:2 * m + 2],
                                    axis=X, op=MAX)
            nc.vector.tensor_scalar(sx[:, mc], mx[:, mc], 1.0 / QMAX, 1e-8,
                                    op0=MULT, op1=MAX)
            nr_recip(ix[:, mc], sx[:, mc], mc)
            for h in range(2):
                t1 = ph1.tile([P, HH], f32, tag="t1", name=f"t1_{m}_{h}")
                nc.scalar.activation(t1[:], xh[h][:], Copy, bias=MAGIC,
                                     scale=ix[:, mc])
                qt = ph1.tile([P, HH], bf16, tag="qt", name=f"qt{m}_{h}")
                nc.gpsimd.tensor_scalar(qt[:], t1[:], MAGIC, None, op0=SUB)
                nc.scalar.dma_start(qT[g][j][:, h * (KH // 2):(h + 1) * (KH // 2), :],
                                    qt[:], transpose=True)

        def phase2_block(g, n):
            """Fused gate||up gemm block n for group g + swiglu epilogue."""
            nb = slice(n * (NB2 // 2), (n + 1) * (NB2 // 2))  # I-cols of this block
            # wgu before sb: the sb slot wait (gated on an epilogue) must not
            # head-of-line-block the weight prefetch on the sync HWDGE ring
            wgu_t = wgu_p.tile([P, KH * NB2], bf16, tag="wgu", name=f"wgu{g}_{n}")
            nc.sync.dma_start(wgu_t[:], wgu_d[n * P:(n + 1) * P, :])
            sb = sgu_p.tile([P, NB2], f32, tag="sb", name=f"sb{g}_{n}")
            nc.sync.dma_start(sb[:], sgu_d[n:n + 1, :].broadcast_to([P, NB2]))
            for j in range(GT):
                m = g * GT + j
                mc = slice(m, m + 1)
                pgu = ps2_p.tile([P, NB2], f32, tag="pgu", name=f"pgu{g}_{n}_{j}")
                for k in range(KH):
                    nc.tensor.matmul(pgu[:], qT[g][j][:, k, :],
                                     wgu_t[:, k * NB2:(k + 1) * NB2],
                                     start=(k == 0), stop=(k == KH - 1))
                gs = ep2_p.tile([P, NB2 // 2], f32, tag="gs", name=f"gs{g}_{n}_{j}")
                us = ep2_p.tile([P, NB2 // 2], f32, tag="us", name=f"us{g}_{n}_{j}")
                nc.vector.scalar_tensor_tensor(gs[:], pgu[:, 0:NB2 // 2], sx[:, mc],
                                               sb[:, 0:NB2 // 2],
                                               op0=MULT, op1=MULT)
                nc.vector.scalar_tensor_tensor(us[:], pgu[:, NB2 // 2:NB2], sx[:, mc],
                                               sb[:, NB2 // 2:NB2],
                                               op0=MULT, op1=MULT)
                slu = ep2_p.tile([P, NB2 // 2], f32, tag="slu", name=f"slu{g}_{n}_{j}")
                nc.scalar.activation(slu[:], gs[:], Silu)
                pr = ep2_p.tile([P, NB2 // 2], f32, tag="pr", name=f"pr{g}_{n}_{j}")
                nc.vector.tensor_tensor(pr[:], slu[:], us[:], op=MULT)
                nc.vector.tensor_scalar(inter[g][j][:, nb], pr[:], LIMIT, -LIMIT,
                                        op0=MIN, op1=MAX)
                # hide group-1's input quant under group-0's gate/up matmuls
                if g == 0 and n < GT and j == 1:
                    quant_tile(1, n)

        late_ctx = ExitStack()
        qiT = [[None] * GT, [None] * GT]

        def requant_tile(g, j):
            m = g * GT + j
            mc = slice(m, m + 1)
            nc.vector.tensor_reduce(mxi[:, mc], inter[g][j][:], axis=X, op=MAX,
                                    apply_absolute_value=True)
            nc.vector.tensor_scalar(si[:, mc], mxi[:, mc], 1.0 / QMAX, 1e-8,
                                    op0=MULT, op1=MAX)
            nr_recip(ii[:, mc], si[:, mc], mc)
            t2 = rq_p.tile([P, I], f32, tag="t2", name=f"t2_{m}")
            nc.scalar.activation(t2[:], inter[g][j][:], Copy, bias=MAGIC,
                                 scale=ii[:, mc])
            qi = rq_p.tile([P, I], bf16, tag="qi", name=f"qi{m}")
            nc.gpsimd.tensor_scalar(qi[:], t2[:], MAGIC, None, op0=SUB)
            nc.scalar.dma_start(qiT[g][j][:], qi[:], transpose=True)

        def phase3_block(g, n):
            nb = slice(n * NB3, (n + 1) * NB3)
            wd_t = wd_p.tile([P, KI * NB3], bf16, tag="wd", name=f"wd{g}_{n}")
            nc.sync.dma_start(wd_t[:], wd_d[n * P:(n + 1) * P, :])
            for j in range(GT):
                m = g * GT + j
                mc = slice(m, m + 1)
                po = ps3_p.tile([P, NB3], f32, tag="po", name=f"po{g}_{n}_{j}")
                for k in range(KI):
                    nc.tensor.matmul(po[:], qiT[g][j][:, k, :],
                                     wd_t[:, k * NB3:(k + 1) * NB3],
                                     start=(k == 0), stop=(k == KI - 1))
                ot = outp_p.tile([P, NB3], f32, tag="ot", name=f"ot{g}_{n}_{j}")
                nc.vector.tensor_scalar(ot[:], po[:], si[:, mc], None, op0=MULT)
                nc.scalar.dma_start(out_d[m * P:(m + 1) * P, nb], ot[:])

        # ---------------- emission ----------------
        for j in range(GT):
            inter[0][j] = inter_pool.tile([P, I], bf16, tag=f"int_{j}",
                                          name=f"int0_{j}")
        for j in range(GT):
            quant_tile(0, j)
        for n in range(NBI):
            phase2_block(0, n)
        ph1_ctx.close()
        qT_ctx[0].close()

        # late pools go into the SBUF range freed by ph1 + qT0
        rq_p = late_ctx.enter_context(tc.tile_pool(name="rq", bufs=1))
        qiT_p = late_ctx.enter_context(tc.tile_pool(name="qiT", bufs=1))
        wd_p = late_ctx.enter_context(tc.tile_pool(name="wd", bufs=3))
        ps3_p = late_ctx.enter_context(tc.tile_pool(name="ps3", bufs=3,
                                                    space=bass.MemorySpace.PSUM))
        outp_p = late_ctx.enter_context(tc.tile_pool(name="outp", bufs=2))

        # group-0 requant (overlaps group-1 gate/up)
        for j in range(GT):
            qiT[0][j] = qiT_p.tile([P, KI, P], bf16, tag=f"qiT0_{j}",
                                   name=f"qiT0_{j}")
        for j in range(GT):
            inter[1][j] = inter_pool.tile([P, I], bf16, tag=f"int_{j}",
                                          name=f"int1_{j}")
        for j in range(GT):
            requant_tile(0, j)

        for n in range(NBI):
            phase2_block(1, n)

        for n in range(NBH):
            phase3_block(0, n)

        # group-1 requant (overlaps group-0 down-proj; separate tags so these
        # transposes don't wait for group-0's last down-proj matmuls)
        for j in range(GT):
            qiT[1][j] = qiT_p.tile([P, KI, P], bf16, tag=f"qiT1_{j}",
                                   name=f"qiT1_{j}")
        for j in range(GT):
            requant_tile(1, j)

        for n in range(NBH):
            phase3_block(1, n)

        late_ctx.close()
        qT_ctx[1].close()
        inter_ctx.close()

    if not nc.is_finalized():
        nc.finalize()
    return nc


def _prep_inputs(x, w_gate, s_wgate, w_up, s_wup, w_down):
    bf16 = ml_dtypes.bfloat16
    x_flat = np.ascontiguousarray(x.reshape(-1, H)).astype(bf16)
    # fused gate||up blocks: [NBI, P, KH, 512] with cols 0:256 = gate, 256:512 = up
    gT = np.asarray(w_gate).T.reshape(KH, P, NBI, NB2 // 2).transpose(2, 1, 0, 3)
    uT = np.asarray(w_up).T.reshape(KH, P, NBI, NB2 // 2).transpose(2, 1, 0, 3)
    wgu = np.concatenate([gT, uT], axis=3).astype(bf16)          # int-valued: exact
    wgu = np.ascontiguousarray(wgu.reshape(NBI * P, KH * NB2))
    dT = np.asarray(w_down).T.reshape(KI, P, NBH, NB3).transpose(2, 1, 0, 3)
    wd = np.ascontiguousarray(dT.astype(bf16).reshape(NBH * P, KI * NB3))
    sgu = np.ascontiguousarray(np.concatenate(
        [np.asarray(s_wgate).reshape(NBI, NB2 // 2),
         np.asarray(s_wup).reshape(NBI, NB2 // 2)], axis=1).astype(np.float32))
    return x_flat, wgu, wd, sgu


def kernel(x, w_gate, s_wgate, w_up, s_wup, w_down, s_wdown,
           inv_gate, inv_up, inv_inter):
    from concourse.bass_utils import run_bass_kernel_spmd

    x_flat, wgu, wd, sgu = _prep_inputs(x, w_gate, s_wgate, w_up, s_wup, w_down)

    if "nc" not in _CACHE:
        _CACHE["nc"] = _build()
    nc = _CACHE["nc"]

    in_maps = []
    for c in range(N_CORES):
        in_maps.append({
            "x": np.ascontiguousarray(x_flat[c * T:(c + 1) * T]),
            "wgu": wgu, "wd": wd, "sgu": sgu,
        })
    res = run_bass_kernel_spmd(nc, in_maps, list(range(N_CORES)))
    _CACHE["last_results"] = res
    _CACHE["last_in_maps"] = in_maps
    out = np.concatenate([res.results[c]["out"] for c in range(N_CORES)], axis=0)
    out = out * np.asarray(s_wdown, dtype=np.float32)[None, :]
    return out.reshape(x.shape).astype(np.float32)


# revision 26
# speedup vs baseline: 1.6712x; 1.6712x over previous
"""Trainium2 Bass kernel for a quantized shared-expert MLP (SwiGLU, int8 dynamic quant).

Computation (per reference):
  x [2,4096,4096] f32 -> flatten [8192, 4096] -> bf16
  per-token int8 dynamic quant of x; int8 gemm vs w_gate/w_up (per-channel int8);
  swiglu with +-10 clip -> bf16; per-token requant; int8 gemm vs w_down; f32 out.

Strategy: data-parallel over the 8192 tokens across 8 NeuronCores (1024
tokens/core), weights replicated, no collectives.  Matmuls run in bf16, which
is exact here: quantized values are integers in [-127,127] (exact in bf16).

v2 layout (vs v1): two-group token pipeline so the PE never idles --
[g0 gate/up][g1 gate/up][g0 down][g1 down] with quant / requant / DMA
transposes of each group hidden under the other group's matmuls.  Gate and up
weight blocks are fused into one [128, 32k, 512] stationary stream (single MM
per k-step, N=512, one PSUM bank), host-pre-laid-out so every weight DMA is
contiguous per partition (line rate).  DMA traffic is split over three rings:
weights on Sync (SP), x loads on GpSimd (SWDGE), transposes + output stores on
Scalar (ACT).  s_wdown is folded in on the host after the kernel.
"""

import numpy as np
import ml_dtypes

H = 4096
I = 2048
P = 128
T = 1024           # tokens per core (8192 / 8)
N_CORES = 8
QMAX = 127.0
LIMIT = 10.0
MAGIC = 12582912.0  # 1.5 * 2**23: fp32 add/sub rounds to nearest-even integer

NT = T // P        # 8 token tiles per core
GRP = 2            # token-tile groups (pipeline stages)
GT = NT // GRP     # tiles per group (4)
KH = H // P        # 32 k-tiles for gate/up
KI = I // P        # 16 k-tiles for down
NB2 = 512          # fused gate||up free-dim block (256 gate + 256 up)
NBI = (2 * I) // NB2   # 8 gate/up blocks
NB3 = 256          # down-proj free-dim block
NBH = H // NB3     # 16 down blocks

_CACHE = {}


def _build():
    import concourse.bass as bass
    import concourse.bacc as bacc
    import concourse.mybir as mybir
    from concourse import tile
    from contextlib import ExitStack

    f32 = mybir.dt.float32
    bf16 = mybir.dt.bfloat16
    X = mybir.AxisListType.X
    MAX = mybir.AluOpType.max
    MIN = mybir.AluOpType.min
    MULT = mybir.AluOpType.mult
    ADD = mybir.AluOpType.add
    SUB = mybir.AluOpType.subtract
    Copy = mybir.ActivationFunctionType.Copy
    Silu = mybir.ActivationFunctionType.Silu

    nc = bacc.Bacc("TRN2", target_bir_lowering=False, debug=False)

    x_d = nc.dram_tensor("x", [T, H], bf16, kind="ExternalInput")
    wgu_d = nc.dram_tensor("wgu", [NBI * P, KH * NB2], bf16, kind="ExternalInput")
    wd_d = nc.dram_tensor("wd", [NBH * P, KI * NB3], bf16, kind="ExternalInput")
    sgu_d = nc.dram_tensor("sgu", [NBI, NB2], f32, kind="ExternalInput")
    out_d = nc.dram_tensor("out", [T, H], f32, kind="ExternalOutput")

    HH = H // 2        # phase-1 half tile width (2048)

    with ExitStack() as ctx:
        tc = ctx.enter_context(tile.TileContext(nc))

        sc_p = ctx.enter_context(tc.tile_pool(name="scales", bufs=1))
        sgu_p = ctx.enter_context(tc.tile_pool(name="sgu", bufs=4))
        ep2_p = ctx.enter_context(tc.tile_pool(name="ep2", bufs=2))
        ps2_p = ctx.enter_context(tc.tile_pool(name="ps2", bufs=4,
                                               space=bass.MemorySpace.PSUM))
        wgu_p = ctx.enter_context(tc.tile_pool(name="wgu", bufs=2))

        # per-token-tile scale columns [P, NT]
        mx = sc_p.tile([P, NT], f32, tag="mx")
        mx2 = sc_p.tile([P, 2 * NT], f32, tag="mx2")
        sx = sc_p.tile([P, NT], f32, tag="sx")     # x quant scale
        ix = sc_p.tile([P, NT], f32, tag="ix")     # 1 / sx
        mxi = sc_p.tile([P, NT], f32, tag="mxi")
        si = sc_p.tile([P, NT], f32, tag="si")     # inter quant scale
        ii = sc_p.tile([P, NT], f32, tag="ii")     # 1 / si
        r0 = sc_p.tile([P, NT], f32, tag="r0")
        r1 = sc_p.tile([P, NT], f32, tag="r1")

        def nr_recip(out_col, in_col, mc):
            # out = 1/in with one Newton step (HW seed alone is not enough
            # for exact round() parity with the reference's division)
            nc.vector.reciprocal(r0[:, mc], in_col)
            nc.vector.tensor_tensor(r1[:, mc], in_col, r0[:, mc], op=MULT)
            nc.vector.tensor_scalar(r1[:, mc], r1[:, mc], -1.0, 2.0, op0=MULT, op1=ADD)
            nc.vector.tensor_tensor(out_col, r0[:, mc], r1[:, mc], op=MULT)

        # Pool stack discipline (LIFO): [ctx..., inter, qT1, qT0, ph1].
        # ph1+qT0 pop right after group-0 gate/up; the late pools
        # (rq, qiT, wd, outp) push into that freed space.
        # inter and qiT tiles are shared between groups via tags: group 1's
        # allocation of the same tag waits for group 0's release, which the
        # pipeline order satisfies naturally.
        inter_ctx = ExitStack()
        inter_pool = inter_ctx.enter_context(tc.tile_pool(name="inter", bufs=1))
        inter = [[None] * GT, [None] * GT]
        qT_ctx = [ExitStack(), ExitStack()]
        # push qT1 first, then qT0 (qT0 pops earlier)
        qT_pool = [None, None]
        qT_pool[1] = qT_ctx[1].enter_context(tc.tile_pool(name="qT1", bufs=1))
        qT_pool[0] = qT_ctx[0].enter_context(tc.tile_pool(name="qT0", bufs=1))
        qT = [[qT_pool[g].tile([P, KH, P], bf16, tag=f"qT{g}_{j}",
                               name=f"qT{g}_{j}") for j in range(GT)]
              for g in range(GRP)]

        ph1_ctx = ExitStack()
        ph1 = ph1_ctx.enter_context(tc.tile_pool(name="ph1", bufs=2))

        def quant_tile(g, j):
            """Load token tile m, dynamic-quant, transpose into qT[g][j]."""
            m = g * GT + j
            mc = slice(m, m + 1)
            xh = [None, None]
            for h in range(2):
                xh[h] = ph1.tile([P, HH], bf16, tag=f"xh{h}", name=f"xh{m}_{h}")
                nc.scalar.dma_start(xh[h][:], x_d[m * P:(m + 1) * P,
                                                  h * HH:(h + 1) * HH])
                nc.vector.tensor_reduce(mx2[:, 2 * m + h:2 * m + h + 1], xh[h][:],
                                        axis=X, op=MAX, apply_absolute_value=True)
            nc.vector.tensor_reduce(mx[:, mc], mx2[:, 2 * m:2 * m + 2],
                                    axis=X, op=MAX)
            nc.vector.tensor_scalar(sx[:, mc], mx[:, mc], 1.0 / QMAX, 1e-8,
                                    op0=MULT, op1=MAX)
            nr_recip(ix[:, mc], sx[:, mc], mc)
            for h in range(2):
                t1 = ph1.tile([P, HH], f32, tag="t1", name=f"t1_{m}_{h}")
                nc.scalar.activation(t1[:], xh[h][:], Copy, bias=MAGIC,
                                     scale=ix[:, mc])
                qt = ph1.tile([P, HH], bf16, tag="qt", name=f"qt{m}_{h}")
                nc.vector.tensor_scalar(qt[:], t1[:], MAGIC, None, op0=SUB)
                nc.scalar.dma_start(qT[g][j][:, h * (KH // 2):(h + 1) * (KH // 2), :],
                                    qt[:], transpose=True)

        def phase2_block(g, n):
            """Fused gate||up gemm block n for group g + swiglu epilogue."""
            nb = slice(n * (NB2 // 2), (n + 1) * (NB2 // 2))  # I-cols of this block
            # wgu before sb: the sb slot wait (gated on an epilogue) must not
            # head-of-line-block the weight prefetch on the sync HWDGE ring
            wgu_t = wgu_p.tile([P, KH * NB2], bf16, tag="wgu", name=f"wgu{g}_{n}")
            nc.sync.dma_start(wgu_t[:], wgu_d[n * P:(n + 1) * P, :])
            sb = sgu_p.tile([P, NB2], f32, tag="sb", name=f"sb{g}_{n}")
            nc.sync.dma_start(sb[:], sgu_d[n:n + 1, :].broadcast_to([P, NB2]))
            for j in range(GT):
                m = g * GT + j
                mc = slice(m, m + 1)
                pgu = ps2_p.tile([P, NB2], f32, tag="pgu", name=f"pgu{g}_{n}_{j}")
                for k in range(KH):
                    nc.tensor.matmul(pgu[:], qT[g][j][:, k, :],
                                     wgu_t[:, k * NB2:(k + 1) * NB2],
                                     start=(k == 0), stop=(k == KH - 1))
                gs = ep2_p.tile([P, NB2 // 2], f32, tag="gs", name=f"gs{g}_{n}_{j}")
                us = ep2_p.tile([P, NB2 // 2], f32, tag="us", name=f"us{g}_{n}_{j}")
                nc.vector.scalar_tensor_tensor(gs[:], pgu[:, 0:NB2 // 2], sx[:, mc],
                                               sb[:, 0:NB2 // 2],
                                               op0=MULT, op1=MULT)
                nc.vector.scalar_tensor_tensor(us[:], pgu[:, NB2 // 2:NB2], sx[:, mc],
                                               sb[:, NB2 // 2:NB2],
                                               op0=MULT, op1=MULT)
                slu = ep2_p.tile([P, NB2 // 2], f32, tag="slu", name=f"slu{g}_{n}_{j}")
                nc.scalar.activation(slu[:], gs[:], Silu)
                pr = ep2_p.tile([P, NB2 // 2], f32, tag="pr", name=f"pr{g}_{n}_{j}")
                nc.vector.tensor_tensor(pr[:], slu[:], us[:], op=MULT)
                nc.vector.tensor_scalar(inter[g][j][:, nb], pr[:], LIMIT, -LIMIT,
                                        op0=MIN, op1=MAX)
                # hide group-1's input quant under group-0's gate/up matmuls
                if g == 0 and n < GT and j == 1:
                    quant_tile(1, n)

        late_ctx = ExitStack()
        qiT = [[None] * GT, [None] * GT]

        def requant_tile(g, j):
            m = g * GT + j
            mc = slice(m, m + 1)
            nc.vector.tensor_reduce(mxi[:, mc], inter[g][j][:], axis=X, op=MAX,
                                    apply_absolute_value=True)
            nc.vector.tensor_scalar(si[:, mc], mxi[:, mc], 1.0 / QMAX, 1e-8,
                                    op0=MULT, op1=MAX)
            nr_recip(ii[:, mc], si[:, mc], mc)
            t2 = rq_p.tile([P, I], f32, tag="t2", name=f"t2_{m}")
            nc.scalar.activation(t2[:], inter[g][j][:], Copy, bias=MAGIC,
                                 scale=ii[:, mc])
            qi = rq_p.tile([P, I], bf16, tag="qi", name=f"qi{m}")
            nc.vector.tensor_scalar(qi[:], t2[:], MAGIC, None, op0=SUB)
            nc.scalar.dma_start(qiT[g][j][:], qi[:], transpose=True)

        def phase3_block(g, n):
            nb = slice(n * NB3, (n + 1) * NB3)
            wd_t = wd_p.tile([P, KI * NB3], bf16, tag="wd", name=f"wd{g}_{n}")
            nc.sync.dma_start(wd_t[:], wd_d[n * P:(n + 1) * P, :])
            for j in range(GT):
                m = g * GT + j
                mc = slice(m, m + 1)
                po = ps3_p.tile([P, NB3], f32, tag="po", name=f"po{g}_{n}_{j}")
                for k in range(KI):
                    nc.tensor.matmul(po[:], qiT[g][j][:, k, :],
                                     wd_t[:, k * NB3:(k + 1) * NB3],
                                     start=(k == 0), stop=(k == KI - 1))
                ot = outp_p.tile([P, NB3], f32, tag="ot", name=f"ot{g}_{n}_{j}")
                nc.vector.tensor_scalar(ot[:], po[:], si[:, mc], None, op0=MULT)
                nc.scalar.dma_start(out_d[m * P:(m + 1) * P, nb], ot[:])

        # ---------------- emission ----------------
        for j in range(GT):
            inter[0][j] = inter_pool.tile([P, I], bf16, tag=f"int_{j}",
                                          name=f"int0_{j}")
        for j in range(GT):
            quant_tile(0, j)
        for n in range(NBI):
            phase2_block(0, n)
        ph1_ctx.close()
        qT_ctx[0].close()

        # late pools go into the SBUF range freed by ph1 + qT0
        rq_p = late_ctx.enter_context(tc.tile_pool(name="rq", bufs=1))
        qiT_p = late_ctx.enter_context(tc.tile_pool(name="qiT", bufs=1))
        wd_p = late_ctx.enter_context(tc.tile_pool(name="wd", bufs=3))
        ps3_p = late_ctx.enter_context(tc.tile_pool(name="ps3", bufs=3,
                                                    space=bass.MemorySpace.PSUM))
        outp_p = late_ctx.enter_context(tc.tile_pool(name="outp", bufs=2))

        # group-0 requant (overlaps group-1 gate/up)
        for j in range(GT):
            qiT[0][j] = qiT_p.tile([P, KI, P], bf16, tag=f"qiT0_{j}",
                                   name=f"qiT0_{j}")
        for j in range(GT):
            inter[1][j] = inter_pool.tile([P, I], bf16, tag=f"int_{j}",
                                          name=f"int1_{j}")
        for j in range(GT):
            requant_tile(0, j)

        for n in range(NBI):
            phase2_block(1, n)

        for n in range(NBH):
            phase3_block(0, n)

        # group-1 requant (overlaps group-0 down-proj; separate tags so these
        # transposes don't wait for group-0's last down-proj matmuls)
        for j in range(GT):
            qiT[1][j] = qiT_p.tile([P, KI, P], bf16, tag=f"qiT1_{j}",
                                   name=f"qiT1_{j}")
        for j in range(GT):
            requant_tile(1, j)

        for n in range(NBH):
            phase3_block(1, n)

        late_ctx.close()
        qT_ctx[1].close()
        inter_ctx.close()

    if not nc.is_finalized():
        nc.finalize()
    return nc


def _prep_inputs(x, w_gate, s_wgate, w_up, s_wup, w_down):
    bf16 = ml_dtypes.bfloat16
    x_flat = np.ascontiguousarray(x.reshape(-1, H)).astype(bf16)
    # fused gate||up blocks: [NBI, P, KH, 512] with cols 0:256 = gate, 256:512 = up
    gT = np.asarray(w_gate).T.reshape(KH, P, NBI, NB2 // 2).transpose(2, 1, 0, 3)
    uT = np.asarray(w_up).T.reshape(KH, P, NBI, NB2 // 2).transpose(2, 1, 0, 3)
    wgu = np.concatenate([gT, uT], axis=3).astype(bf16)          # int-valued: exact
    wgu = np.ascontiguousarray(wgu.reshape(NBI * P, KH * NB2))
    dT = np.asarray(w_down).T.reshape(KI, P, NBH, NB3).transpose(2, 1, 0, 3)
    wd = np.ascontiguousarray(dT.astype(bf16).reshape(NBH * P, KI * NB3))
    sgu = np.ascontiguousarray(np.concatenate(
        [np.asarray(s_wgate).reshape(NBI, NB2 // 2),
         np.asarray(s_wup).reshape(NBI, NB2 // 2)], axis=1).astype(np.float32))
    return x_flat, wgu, wd, sgu


def kernel(x, w_gate, s_wgate, w_up, s_wup, w_down, s_wdown,
           inv_gate, inv_up, inv_inter):
    from concourse.bass_utils import run_bass_kernel_spmd

    x_flat, wgu, wd, sgu = _prep_inputs(x, w_gate, s_wgate, w_up, s_wup, w_down)

    if "nc" not in _CACHE:
        _CACHE["nc"] = _build()
    nc = _CACHE["nc"]

    in_maps = []
    for c in range(N_CORES):
        in_maps.append({
            "x": np.ascontiguousarray(x_flat[c * T:(c + 1) * T]),
            "wgu": wgu, "wd": wd, "sgu": sgu,
        })
    res = run_bass_kernel_spmd(nc, in_maps, list(range(N_CORES)))
    _CACHE["last_results"] = res
    _CACHE["last_in_maps"] = in_maps
    out = np.concatenate([res.results[c]["out"] for c in range(N_CORES)], axis=0)
    out = out * np.asarray(s_wdown, dtype=np.float32)[None, :]
    return out.reshape(x.shape).astype(np.float32)
